# revision 28
# baseline (speedup 1.0000x reference)
"""Multi-head attention (B=2, S=2048, H=1024, NH=16 heads of 64) on 8 trn2
NeuronCores, tensor-parallel over heads with batch parallelism on top.

Sharding: core c handles batch b=c//4 and head-group g=c%4 (4 heads, 256 of
the 1024 hidden cols). Each core computes the partial output
ctx_g @ Wo[g_rows, :]; the host sums the 4 partials per batch and adds the
closed-form bias terms (bv @ Wo + bo; bq/bk are applied on-device).

Device math (per core), in transposed-score space. The PE cost model charges
matmuls by output columns only, so the wins come from fp8 DoubleRow mode
(0.5 cycles/col, contraction 2x128 via paired k-tiles in the free dim):

  projections: x and W are split hi/lo into fp8 on the host (x = hi + lo,
    W*32 = hi + lo); each projection runs 3 DoubleRow passes
    (hi*hi + lo*hi + hi*lo) per 256-row chunk pair -- 25% fewer PE cycles
    than fp16 at ~1e-3 relative error (the dropped lo*lo term is ~0.1%).
  scores: q/k are drained to BOTH fp16 tiles (baseline layout) and fp8
    "zero-plane" tiles q8z/k8z [128, 2, S] (k-tile plane 1 is zeros so a
    64-dim head contracts correctly in DoubleRow). The first RHO8 of 4
    key 512-chunks use 1-pass fp8 DR scores (half the PE cycles); the rest
    use fp16. RHO8 trades rel-err (~1.76e-2 * sqrt(RHO8/4)) for PE time.
  exp on ACT: scale 0.125/1024 (q,k carry x32 each), bias = mask.
  ctx/normalize/output projection: fp16 (fp8 here fails the 2e-2 gate).

  exp: mostly on ACT (scale 0.125/1024, bias = mask); 3 of 16 kc per
    phase-2 combo instead use a Schraudolph fp16-bits exp on DVE
    (u16 = psum*A + B reinterpreted as fp16, ~1.8% rms multiplicative
    noise that the softmax denominator largely cancels), because ACT is
    the phase-2 pace-setter and DVE has slack.

Schedule: phase 1 interleaves projections with attention on the first
q-chunk of BOTH head pairs (2 ctx accumulators); phase 2 runs one combo at
a time with norms/tails/q-projections spliced in as PE filler. GPSIMD
cannot touch PSUM on TRN2, so every drain is on DVE (or ACT in the final
tail). Output DMA is fp16; the host sums partials in fp32.

Measured (cost model = graded metric): 193145 ns; rel err 1.54e-2.
"""

import os
import sys

sys.path.insert(0, "/opt/trn_rl_repo")

import numpy as np

B, S, H, NH, HD = 2, 2048, 1024, 16, 64
NCORES = 8
HPC = 4          # heads per core
COLS = HPC * HD  # 256
KC = S // 128    # 16 k chunks
QB = 1024        # q block width
NQT = S // 128   # 16 global q tiles
SC = 512         # seq chunk for projections
RHO8 = 3         # of 4 key 512-chunks computed with fp8 DR scores

_CACHE = {}


def _build():
    import concourse.mybir as mybir
    import concourse.tile as tile
    from concourse import bacc
    from concourse.masks import make_identity

    f32 = mybir.dt.float32
    f16 = mybir.dt.float16
    f8 = mybir.dt.float8e4
    u16 = mybir.dt.uint16
    Exp = mybir.ActivationFunctionType.Exp
    DR = mybir.MatmulPerfMode.DoubleRow

    nc = bacc.Bacc("TRN2", target_bir_lowering=False, debug=False,
                   num_devices=NCORES)

    xhi_d = nc.dram_tensor("xhi", [H, S], f8, kind="ExternalInput").ap()
    xlo_d = nc.dram_tensor("xlo", [H, S], f8, kind="ExternalInput").ap()
    # weights arrive pre-packed in the DoubleRow SBUF layout:
    # w[p, c*1024 + a*256 + n] = (W*32)[(a*2+c)*128 + p, n]
    wq_h = nc.dram_tensor("wqh", [128, 2048], f8, kind="ExternalInput").ap()
    wq_l = nc.dram_tensor("wql", [128, 2048], f8, kind="ExternalInput").ap()
    wk_h = nc.dram_tensor("wkh", [128, 2048], f8, kind="ExternalInput").ap()
    wk_l = nc.dram_tensor("wkl", [128, 2048], f8, kind="ExternalInput").ap()
    wv_h = nc.dram_tensor("wvh", [128, 2048], f8, kind="ExternalInput").ap()
    wv_l = nc.dram_tensor("wvl", [128, 2048], f8, kind="ExternalInput").ap()
    wo_d = nc.dram_tensor("wo", [COLS, H], f16, kind="ExternalInput").ap()
    bq_d = nc.dram_tensor("bq", [COLS], f32, kind="ExternalInput").ap()
    bk_d = nc.dram_tensor("bk", [COLS], f32, kind="ExternalInput").ap()
    mask_d = nc.dram_tensor("mask", [S], f32, kind="ExternalInput").ap()
    out_d = nc.dram_tensor("out", [S, H], f16, kind="ExternalOutput").ap()

    with tile.TileContext(nc) as tc:
        pers = tc.alloc_tile_pool(name="pers", bufs=1)
        psA = tc.alloc_tile_pool(name="psA", bufs=2, space="PSUM")
        psB = tc.alloc_tile_pool(name="psB", bufs=2, space="PSUM")
        work = tc.alloc_tile_pool(name="work", bufs=3)

        # fp16 q/k (baseline layout, used by the fp16-score kc chunks)
        qT = [pers.tile([128, S], f16, tag=f"qT{i}", name=f"qT{i}")
              for i in range(2)]
        kT = [pers.tile([128, S], f16, tag=f"kT{i}", name=f"kT{i}")
              for i in range(2)] if RHO8 < 4 else None
        # fp8 zero-plane q/k for DoubleRow scores: per head-pair [128, 2, S],
        # k-tile plane 1 = 0 so the 64-dim contraction is exact
        q8z = [pers.tile([128, 2, S], f8, tag=f"q8z{i}", name=f"q8z{i}")
               for i in range(2)]
        k8z = [pers.tile([128, 2, S], f8, tag=f"k8z{i}", name=f"k8z{i}")
               for i in range(2)]
        vt = [pers.tile([128, HPC * 65], f16, tag=f"v{i}", name=f"v{i}")
              for i in range(KC)]
        asm = [pers.tile([128, COLS], f16, tag=f"asm{i}", name=f"asm{i}")
               for i in range(NQT)]
        # x hi/lo in DoubleRow layout: tile t holds rows [256t, 256t+256)
        # as [128, 2, S] (k-tile pair in dim 1)
        xt8h = [pers.tile([128, 2, S], f8, tag=f"x8h{i}", name=f"x8h{i}")
                for i in range(4)]
        xt8l = [pers.tile([128, 2, S], f8, tag=f"x8l{i}", name=f"x8l{i}")
                for i in range(4)]
        # weights hi/lo: [128, 2, 4*256] (hcp-major free dim)
        wq8 = [pers.tile([128, 2, 1024], f8, tag=f"wq8{i}", name=f"wq8{i}")
               for i in range(2)]
        wk8 = [pers.tile([128, 2, 1024], f8, tag=f"wk8{i}", name=f"wk8{i}")
               for i in range(2)]
        wv8 = [pers.tile([128, 2, 1024], f8, tag=f"wv8{i}", name=f"wv8{i}")
               for i in range(2)]
        wo_a = pers.tile([128, 2048], f16, tag="wo", name="wo_a")

        bq_s = pers.tile([128, 2], f32, tag="bq", name="bq_s")
        bk_s = pers.tile([128, 2], f32, tag="bk", name="bk_s")
        mask_s = pers.tile([128, KC], f32, tag="mask", name="mask_s")
        mask16 = pers.tile([128, KC], f32, tag="mask16", name="mask16")
        id65 = pers.tile([65, 65], f16, tag="id65", name="id65")
        id128 = pers.tile([128, 128], f16, tag="id128", name="id128")

        warm = pers.tile([1, 1], f32, tag="warm", name="warm")
        nc.gpsimd.memset(warm[:], 0.0)
        nc.scalar.activation(warm[:], warm[:], Exp)
        make_identity(nc, id65[:])
        make_identity(nc, id128[:])
        # zero planes for the fp8 DoubleRow operands
        for i in range(2):
            nc.gpsimd.memset(q8z[i][:, 1, :], 0.0)
            nc.gpsimd.memset(k8z[i][:, 1, :], 0.0)
        # aug columns of v (denominator accumulators), set once
        for st in range(KC):
            nc.gpsimd.memset(
                vt[st].rearrange("p (h c) -> p h c", c=65)[:, :, 64:65], 1.0)

        # Few large DMAs on one HWDGE queue, ordered so the projection
        # pipeline starts as early as possible (queue order = arrival order).
        def x_pair(t, lo, hi, which):
            dst = (xt8h if which == 0 else xt8l)[t][:, :, lo:hi]
            src = (xhi_d if which == 0 else xlo_d)[
                t * 256:(t + 1) * 256, lo:hi].rearrange("(c p) s -> p c s",
                                                        p=128)
            nc.sync.dma_start(dst, src)

        def w_load(dst, src):
            nc.sync.dma_start(dst[:].rearrange("p c n -> p (c n)"), src)

        w_load(wq8[0], wq_h)
        for t in range(4):
            x_pair(t, 0, SC, 0)
        w_load(wk8[0], wk_h)
        for t in range(4):
            x_pair(t, 0, SC, 1)
        w_load(wq8[1], wq_l)
        w_load(wk8[1], wk_l)
        w_load(wv8[0], wv_h)
        w_load(wv8[1], wv_l)
        nc.sync.dma_start(bq_s[:], bq_d.rearrange("(a p) -> p a", p=128))
        nc.sync.dma_start(bk_s[:], bk_d.rearrange("(a p) -> p a", p=128))
        nc.sync.dma_start(mask_s[:], mask_d.rearrange("(a p) -> p a", p=128))
        for t in range(4):
            x_pair(t, SC, S, 0)
            x_pair(t, SC, S, 1)
        nc.sync.dma_start(wo_a.rearrange("p (c n) -> p c n", c=2),
                          wo_d.rearrange("(c p) n -> p c n", p=128))

        Ident = mybir.ActivationFunctionType.Identity
        # Schraudolph fp16-bits exp on DVE: u16 = psum*A + B(mask) with the
        # uint16 result reinterpreted as fp16 (~1.6% rms multiplicative
        # error; softmax denominators stay consistent). Used for a few kc
        # per phase-2 combo to take load off ACT, which paces phase 2.
        SCHRAU_A = 1024.0 * 1.4426950408889634 / 8192.0
        nc.vector.tensor_scalar(mask16[:], mask_s[:],
                                1024.0 * 1.4426950408889634,
                                15360.0 - 33.0,
                                mybir.AluOpType.mult, mybir.AluOpType.add)
        DVE_KCS = (4, 9, 14)

        def fp8_chunk(kc):
            return kc < 4 * RHO8

        def proj_mms(ps, w8, pi, sc, n0, n1):
            """12 DoubleRow matmuls: (hi,hi),(lo,hi),(hi,lo) x 4 hcp."""
            first = True
            for xa, wb in ((0, 0), (1, 0), (0, 1)):
                xs = xt8h if xa == 0 else xt8l
                for hcp in range(4):
                    last = (xa == 0 and wb == 1 and hcp == 3)
                    nc.tensor.matmul(
                        ps[:],
                        w8[wb][:, :, hcp * 256 + pi * 128:
                               hcp * 256 + pi * 128 + 128],
                        xs[hcp][:, :, n0:n1],
                        start=first, stop=last, perf_mode=DR)
                    first = False

        def qk_proj(w8, b_s, dst16, dst8, pi, sc, eng="vector"):
            """Project q or k for (pi, sc); drain to fp16 and/or fp8 tiles.
            eng: 'act' uses the scalar engine (phase 1), else DVE/Pool."""
            ps = psA.tile([128, SC], f32, tag="sc", name="pps")
            proj_mms(ps, w8, pi, sc, sc * SC, (sc + 1) * SC)
            sl = slice(sc * SC, (sc + 1) * SC)
            outs = []
            if dst16 is not None:
                outs.append(dst16[pi][:, sl])
            if dst8 is not None:
                outs.append(dst8[pi][:, 0, sl])
            for o in outs:
                if eng == "act":
                    nc.scalar.activation(o, ps[:], Ident,
                                         bias=b_s[:, pi:pi + 1])
                else:
                    nc.vector.tensor_scalar_add(o, ps[:], b_s[:, pi:pi + 1])

        def v_proj2(st, eng="vector"):
            ps = psA.tile([128, COLS], f32, tag="sc", name="vps")
            first = True
            for xa, wb in ((0, 0), (1, 0), (0, 1)):
                xs = xt8h if xa == 0 else xt8l
                for hcp in range(4):
                    last = (xa == 0 and wb == 1 and hcp == 3)
                    nc.tensor.matmul(
                        ps[:],
                        xs[hcp][:, :, st * 128:(st + 1) * 128],
                        (wv8[wb])[:, :, hcp * 256:(hcp + 1) * 256],
                        start=first, stop=last, perf_mode=DR)
                    first = False
            dst = vt[st].rearrange("p (h c) -> p h c", c=65)[:, :, 0:64]
            src = ps[:].rearrange("p (h c) -> p h c", c=64)
            if eng == "act":
                nc.scalar.activation(dst, src, Ident)
            else:
                nc.vector.tensor_copy(dst, src)

        # ---- attention machinery ----
        ctx_open = {}   # (hp, qb4) -> open PSUM accumulator
        ctx_done = {}   # (hp, qb4) -> SBUF ctx ready for normalize
        pend = []       # global pending ctx matmuls (software pipeline)
        pend_cap = [8]  # mutable flush threshold (phase-dependent)

        def emit_ctx(ctx_ps, hp, kc, ex):
            for j in range(2):
                h = hp * 2 + j
                nc.tensor.matmul(ctx_ps[:, j * 512:(j + 1) * 512],
                                 vt[kc][:, h * 65:(h + 1) * 65],
                                 ex[:, j * 512:(j + 1) * 512],
                                 start=(kc == 0), stop=(kc == KC - 1))

        def attn(hp, qb4, kcs, dve_exp=False):
            """Emit scores+exp for the given kcs of combo (hp, qb4); ctx
            matmuls are deferred through a global pipeline so the in-order
            PE never waits on the exp they consume."""
            key = (hp, qb4)
            if key not in ctx_open:
                ctx_open[key] = psB.tile([65, QB], f32, tag="cx",
                                         name=f"ctx{hp}_{qb4}")
            ctx_ps = ctx_open[key]
            qs = qb4 * 512
            for kc in kcs:
                sc_ps = psA.tile([128, QB], f32, tag="sc", name="sc_ps")
                for j in range(2):
                    if fp8_chunk(kc):
                        nc.tensor.matmul(
                            sc_ps[:, j * 512:(j + 1) * 512],
                            k8z[hp][64 * j:64 * j + 64, :,
                                    kc * 128:(kc + 1) * 128],
                            q8z[hp][64 * j:64 * j + 64, :, qs:qs + 512],
                            start=True, stop=True, perf_mode=DR)
                    else:
                        nc.tensor.matmul(
                            sc_ps[:, j * 512:(j + 1) * 512],
                            kT[hp][j * 64:j * 64 + 64,
                                   kc * 128:(kc + 1) * 128],
                            qT[hp][j * 64:j * 64 + 64, qs:qs + 512],
                            start=True, stop=True)
                ex = work.tile([128, QB], f16, tag="exp", name="exp", bufs=24)
                if dve_exp and kc in DVE_KCS:
                    nc.vector.tensor_scalar(
                        ex[:].bitcast(u16), sc_ps[:], SCHRAU_A,
                        mask16[:, kc:kc + 1],
                        mybir.AluOpType.mult, mybir.AluOpType.add)
                else:
                    nc.scalar.activation(ex[:], sc_ps[:], Exp,
                                         bias=mask_s[:, kc:kc + 1],
                                         scale=0.125 / 1024.0)
                while len(pend) >= pend_cap[0]:
                    emit_ctx(*pend.pop(0))
                pend.append((ctx_ps, hp, kc, ex))

        def finish_copy(hp, qb4):
            ctx_ps = ctx_open.pop((hp, qb4))
            for it in [p for p in pend if p[0] is ctx_ps]:
                pend.remove(it)
                emit_ctx(*it)
            ctx_sb = work.tile([65, QB], f16, tag="ctxsb", name="ctx_sb",
                               bufs=4)
            nc.vector.tensor_copy(ctx_sb[:], ctx_ps[:])
            ctx_done[(hp, qb4)] = ctx_sb

        def finish_norm(hp, qb4):
            ctx_sb = ctx_done.pop((hp, qb4))
            for j in range(2):
                h = hp * 2 + j
                # stride-68 blocks keep each qt transpose 4-byte aligned
                # in PSUM (65 fp16 = 130 B is not)
                t1p = psB.tile([128, 272], f16, tag="cx", name="t1p")
                for qt in range(4):
                    nc.tensor.transpose(
                        t1p[:, qt * 68:qt * 68 + 65],
                        ctx_sb[:, j * 512 + qt * 128:j * 512 + (qt + 1) * 128],
                        id65[:])
                rc4 = work.tile([128, 4], f32, tag="rc", name="rc")
                nc.vector.reciprocal(
                    rc4[:], t1p.rearrange("p (q c) -> p q c", c=68)[:, :, 64])
                for qt in range(4):
                    nc.vector.tensor_scalar_mul(
                        asm[qb4 * 4 + qt][:, h * 64:(h + 1) * 64],
                        t1p[:, qt * 68:qt * 68 + 64], rc4[:, qt:qt + 1])

        def tail(qb4, qts=range(4), act=False):
            cp_ctn = nc.scalar.copy if act else nc.vector.tensor_copy
            for qt in qts:
                gqt = qb4 * 4 + qt
                t2p = psB.tile([128, 256], f16, tag="cx", name="t2p")
                for cc in range(2):
                    nc.tensor.transpose(
                        t2p[:, cc * 128:(cc + 1) * 128],
                        asm[gqt][:, cc * 128:(cc + 1) * 128], id128[:])
                ctn = work.tile([128, 256], f16, tag="ctn", name="ctn", bufs=4)
                cp_ctn(ctn[:], t2p[:])
                op = psB.tile([128, H], f32, tag="cx", name="op")
                for cc in range(2):
                    for fj in range(2):
                        nc.tensor.matmul(op[:, fj * 512:(fj + 1) * 512],
                                         ctn[:, cc * 128:(cc + 1) * 128],
                                         wo_a[:, cc * H + fj * 512:
                                              cc * H + (fj + 1) * 512],
                                         start=(cc == 0), stop=(cc == 1))
                ob = work.tile([128, H], f16, tag="ob", name="ob", bufs=4)
                if act and qt % 2 == 0:
                    nc.scalar.copy(ob[:], op[:])
                else:
                    nc.vector.tensor_copy(ob[:], op[:])
                nc.sync.dma_start(out_d[gqt * 128:(gqt + 1) * 128, :], ob[:])

        def proj_q(sc):
            for pi in range(2):
                qk_proj(wq8, bq_s, qT, q8z, pi, sc)

        # ---- schedule ----
        # Phase 1: projections at single-kc interleave with attention on the
        # first q-chunk for BOTH head pairs (2 ctx accumulators in psB; v
        # psums ride the psA rotation). All drains go to DVE/Pool so ACT is
        # exp-only from the start. q projections for q-chunks 2-3 are
        # deferred into phase 2 as PE filler while ACT is the bottleneck.
        def kdsts(sc):
            k16 = kT if (kT is not None and not fp8_chunk(sc * 4)) else None
            k8 = k8z if fp8_chunk(sc * 4) else None
            return k16, k8

        k16, k8 = kdsts(0)
        qk_proj(wq8, bq_s, qT, q8z, 0, 0)
        qk_proj(wk8, bk_s, k16, k8, 0, 0)
        v_proj2(0)
        attn(0, 0, [0])
        qk_proj(wq8, bq_s, qT, q8z, 1, 0)
        qk_proj(wk8, bk_s, k16, k8, 1, 0)
        attn(1, 0, [0])
        for kc in range(1, 4):
            v_proj2(kc)
            attn(0, 0, [kc])
            attn(1, 0, [kc])
        for sc in range(1, 4):
            k16, k8 = kdsts(sc)
            for pi in range(2):
                qk_proj(wk8, bk_s, k16, k8, pi, sc)
            for i in range(4):
                kc = sc * 4 + i
                v_proj2(kc)
                attn(0, 0, [kc])
                attn(1, 0, [kc])
        proj_q(1)

        # Phase 2: one head-pair in flight at a time; fillers spliced into
        # each combo's kc loop while ACT chews through exp.
        def emit_item(it):
            kind, arg = it
            if kind == "n":
                finish_norm(*arg)
            elif kind == "t":
                tail(arg[0], qts=[arg[1]])
            else:
                for pi in range(2):
                    qk_proj(wq8, bq_s, qT, q8z, pi, arg[1])

        plan = [
            ((0, 1), [("n", (0, 0)), ("n", (1, 0)), ("q", (0, 2))]),
            ((1, 1), [("t", (0, 0)), ("t", (0, 1)), ("t", (0, 2)),
                      ("t", (0, 3))]),
            ((0, 2), [("n", (0, 1)), ("n", (1, 1)), ("q", (0, 3))]),
            ((1, 2), [("t", (1, 0)), ("t", (1, 1)), ("t", (1, 2)),
                      ("t", (1, 3))]),
            ((0, 3), [("n", (0, 2)), ("n", (1, 2)), ("t", (2, 0)),
                      ("t", (2, 1))]),
            ((1, 3), [("t", (2, 2)), ("t", (2, 3)), ("n", (0, 3))]),
        ]
        bounds = [(0, 2), (2, 4), (4, 8), (8, 12), (12, 14), (14, KC)]
        finish_copy(0, 0)
        prev = (1, 0)
        for (hp, qb4), items in plan:
            cur = 0
            for i, (k0, k1) in enumerate(bounds):
                attn(hp, qb4, range(k0, k1), dve_exp=True)
                if i == 1 and prev is not None:
                    finish_copy(*prev)
                if i >= 2 and cur < len(items):
                    emit_item(items[cur])
                    cur += 1
            while cur < len(items):
                emit_item(items[cur])
                cur += 1
            prev = (hp, qb4)
        finish_copy(1, 3)
        finish_norm(1, 3)
        tail(3, act=True)

        work.release()
        psB.release()
        psA.release()
        pers.release()

    nc.compile()
    return nc


def _get_nc():
    if "nc" not in _CACHE:
        _CACHE["nc"] = _build()
    return _CACHE["nc"]


def kernel(hidden_states, attention_mask, Wq, bq, Wk, bk, Wv, bv, Wo, bo):
    import ml_dtypes
    from concourse.bass_utils import run_bass_kernel_spmd

    F8 = ml_dtypes.float8_e4m3fn

    hidden_states = np.asarray(hidden_states, np.float32)
    attention_mask = np.asarray(attention_mask, np.float32)
    Wq, Wk, Wv, Wo = (np.asarray(a, np.float32) for a in (Wq, Wk, Wv, Wo))
    bq, bk, bv, bo = (np.asarray(a, np.float32) for a in (bq, bk, bv, bo))

    nc = _get_nc()

    def hilo(a):
        hi = np.ascontiguousarray(a).astype(F8)
        lo = (a - hi.astype(np.float32)).astype(F8)
        return hi, lo

    xT = [np.ascontiguousarray(hidden_states[b].T) for b in range(B)]
    xhl = [hilo(x) for x in xT]
    maskb = [np.ascontiguousarray(attention_mask[b, 0, 0, :])
             for b in range(B)]
    in_maps = []
    for c in range(NCORES):
        b, g = c // HPC, c % HPC
        cs = slice(g * COLS, (g + 1) * COLS)
        def wpack(a):
            # [1024, 256] -> (a,c,p,n) -> (p,c,a,n) -> [128, 2048]
            return np.ascontiguousarray(
                a.reshape(4, 2, 128, COLS).transpose(2, 1, 0, 3)
                .reshape(128, 2048))

        wqh, wql = hilo(Wq[:, cs] * 32.0)
        wkh, wkl = hilo(Wk[:, cs] * 32.0)
        wvh, wvl = hilo(Wv[:, cs] * 32.0)
        wqh, wql, wkh, wkl, wvh, wvl = (
            wpack(w) for w in (wqh, wql, wkh, wkl, wvh, wvl))
        in_maps.append({
            "xhi": xhl[b][0], "xlo": xhl[b][1],
            "wqh": wqh, "wql": wql,
            "wkh": wkh, "wkl": wkl,
            "wvh": wvh, "wvl": wvl,
            "wo": np.ascontiguousarray(Wo[cs, :] / 32.0).astype(np.float16),
            "bq": np.ascontiguousarray(bq[cs] * 32.0),
            "bk": np.ascontiguousarray(bk[cs] * 32.0),
            "mask": maskb[b],
        })

    trace = bool(os.environ.get("KERNEL_TRACE"))
    kw = {}
    if trace:
        kw = dict(trace=True, tmpdir=os.environ.get("KERNEL_TRACE_DIR"))
    res = run_bass_kernel_spmd(nc, in_maps, list(range(NCORES)), **kw)
    _CACHE["last_result"] = res

    out = np.zeros((B, S, H), np.float32)
    for c in range(NCORES):
        out[c // HPC] += res.results[c]["out"].astype(np.float32)
    out += bv @ Wo + bo
    return out
